# revision 11
# baseline (speedup 1.0000x reference)
"""Trainium2 Bass kernel for nn_BoxDetectionLoss (8-core data parallel).

Math: reference loss = sum_{a,r,c}[ has_match ? coord+conf_loss : conf^2 ] / denom.
A pixel (r,c) can only match a target box t if r==tb[t,0] and c==tb[t,1]
(T=16 boxes per image), so the dense term is just sum sigmoid(conf_ch)^2 over
channels {2,5,8}; the match term is a correction at <=16 pixels x 3 anchors,
computed from 144 gathered elements per image.

Each of the 8 cores handles one batch image:
  - DMA the 3 conf channels (3MB), ACT sigmoid, DVE fused square+reduce.
  - indirect-DMA gather of pol[ch, tb0[t], tb1[t]] for all t, ch.
  - tiny [16,*] vector ops: pred = clip(tb + sigmoid*scale), round-half-even
    via +/- 1.5*2^23 trick, match compare, first-duplicate mask, correction
    contribution (coord + tp*(tp-2c) which equals coord + (c-tp)^2 - c^2).
  - per-partition accumulator [128, cols] -> free-dim reduce -> [128] output.
Host sums the 8x128 partials and divides by denom.
"""

import numpy as np

B, C, H, W = 8, 9, 512, 512
T = 16
N_CORES = 8
CONF_CH = (2, 5, 8)
DENOM = float(B * H * W * 3)
MAGIC = 12582912.0  # 1.5 * 2^23: x+MAGIC-MAGIC rounds to nearest-even int
FSPLIT = 1          # full-channel tiles
NDENSE = len(CONF_CH)

TRI_CONST = np.tril(np.ones((T, T), dtype=np.float32), -1)  # [t, t'] = t' < t
CHOFF_CONST = np.broadcast_to(
    (np.arange(C, dtype=np.float32) * (H * W))[None, :], (T, C)
).copy()

_PROG = None


def _build_correction(nc, sp, ACC, bass, mybir, tb, tp, tri, choff, pol,
                      gather=True, bcast=True):
    f32 = mybir.dt.float32
    i32 = mybir.dt.int32
    ALU = mybir.AluOpType
    ACT_F = mybir.ActivationFunctionType

    TB = sp.tile([T, 4], i32)
    nc.gpsimd.dma_start(TB[:], tb[:])
    TP = sp.tile([T, 1], f32)
    nc.gpsimd.dma_start(TP[:], tp[:])
    TRI = sp.tile([T, T], f32)
    nc.gpsimd.dma_start(TRI[:], tri[:])
    CH = sp.tile([T, C], f32)
    nc.gpsimd.dma_start(CH[:], choff[:])
    TBrep = sp.tile([T, 4 * T], i32)  # whole tb replicated per row
    if bcast:
        nc.gpsimd.dma_start(
            TBrep[:], tb.rearrange("t f -> (t f)").partition_broadcast(T)
        )
    else:
        nc.vector.memset(TBrep[:], 0)

    TBf = sp.tile([T, 4], f32)
    nc.vector.tensor_copy(TBf[:], TB[:])
    TBrepf = sp.tile([T, 4 * T], f32)
    nc.vector.tensor_copy(TBrepf[:], TBrep[:])

    # packed coords: p1 = r*512 + c, p2 = r2*512 + c2 (exact in f32)
    p1 = sp.tile([T, 1], f32)
    nc.vector.tensor_scalar(
        out=p1[:], in0=TBf[:, 0:1], scalar1=512.0, scalar2=TBf[:, 1:2],
        op0=ALU.mult, op1=ALU.add,
    )
    p2 = sp.tile([T, 1], f32)
    nc.vector.tensor_scalar(
        out=p2[:], in0=TBf[:, 2:3], scalar1=512.0, scalar2=TBf[:, 3:4],
        op0=ALU.mult, op1=ALU.add,
    )

    # row-layout packed coords of all boxes, from the replicated copy
    rep4 = TBrepf[:].rearrange("p (t f) -> p f t", f=4)
    p1row = sp.tile([T, T], f32)
    nc.vector.tensor_scalar(
        out=p1row[:], in0=rep4[:, 0, :], scalar1=512.0, scalar2=None,
        op0=ALU.mult,
    )
    nc.vector.tensor_tensor(
        out=p1row[:], in0=p1row[:], in1=rep4[:, 1, :], op=ALU.add
    )
    p2row = sp.tile([T, T], f32)
    nc.vector.tensor_scalar(
        out=p2row[:], in0=rep4[:, 2, :], scalar1=512.0, scalar2=None,
        op0=ALU.mult,
    )
    nc.vector.tensor_tensor(
        out=p2row[:], in0=p2row[:], in1=rep4[:, 3, :], op=ALU.add
    )

    # duplicate-box detection: S[t,t'] = (p1 equal) & (p2 equal), t' < t
    S = sp.tile([T, T], f32)
    nc.vector.tensor_scalar(
        out=S[:], in0=p1row[:], scalar1=p1[:], scalar2=None, op0=ALU.is_equal
    )
    S2 = sp.tile([T, T], f32)
    nc.vector.tensor_scalar(
        out=S2[:], in0=p2row[:], scalar1=p2[:], scalar2=None, op0=ALU.is_equal
    )
    nc.vector.tensor_tensor(out=S[:], in0=S[:], in1=S2[:], op=ALU.mult)
    nc.vector.tensor_tensor(out=S[:], in0=S[:], in1=TRI[:], op=ALU.mult)
    dupc = sp.tile([T, 1], f32)
    nc.vector.tensor_reduce(
        out=dupc[:], in_=S[:], axis=mybir.AxisListType.X, op=ALU.add
    )
    keep = sp.tile([T, 1], f32)
    nc.vector.tensor_scalar(
        out=keep[:], in0=dupc[:], scalar1=0.0, scalar2=None, op0=ALU.is_equal
    )

    # gather pol[ch, tb0[t], tb1[t]] for all (t, ch): offsets = ch*H*W + p1
    OFFf = sp.tile([T, C], f32)
    nc.vector.tensor_scalar(
        out=OFFf[:], in0=CH[:], scalar1=p1[:], scalar2=None, op0=ALU.add
    )
    OFFi = sp.tile([T, C], i32)
    nc.vector.tensor_copy(OFFi[:], OFFf[:])
    G = sp.tile([T, C], f32)
    if gather:
        nc.gpsimd.indirect_dma_start(
            out=G[:], out_offset=None,
            in_=pol.rearrange("c h (w a) -> (c h w) a", a=1),
            in_offset=bass.IndirectOffsetOnAxis(ap=OFFi[:], axis=0),
        )
    else:
        nc.vector.memset(G[:], 0.0)
    GS = sp.tile([T, C], f32)
    nc.scalar.activation(GS[:], G[:], ACT_F.Sigmoid)
    # channel ch = 3a + k: k=0 delta_r, k=1 delta_c, k=2 conf
    gs3 = GS[:].rearrange("p (a k) -> p k a", k=3)

    # pred = clip(tb + sigmoid*scale, 0, 511), all 3 anchors at once
    predr = sp.tile([T, 3], f32)
    nc.vector.tensor_scalar(
        out=predr[:], in0=gs3[:, 0, :], scalar1=9.0, scalar2=TBf[:, 0:1],
        op0=ALU.mult, op1=ALU.add,
    )
    nc.vector.tensor_scalar(
        out=predr[:], in0=predr[:], scalar1=511.0, scalar2=0.0,
        op0=ALU.min, op1=ALU.max,
    )
    predc = sp.tile([T, 3], f32)
    nc.vector.tensor_scalar(
        out=predc[:], in0=gs3[:, 1, :], scalar1=16.0, scalar2=TBf[:, 1:2],
        op0=ALU.mult, op1=ALU.add,
    )
    nc.vector.tensor_scalar(
        out=predc[:], in0=predc[:], scalar1=511.0, scalar2=0.0,
        op0=ALU.min, op1=ALU.max,
    )

    # round to nearest-even integer: (x + 1.5*2^23) - 1.5*2^23
    rr = sp.tile([T, 3], f32)
    nc.vector.tensor_scalar(
        out=rr[:], in0=predr[:], scalar1=MAGIC, scalar2=None, op0=ALU.add
    )
    nc.vector.tensor_scalar(
        out=rr[:], in0=rr[:], scalar1=MAGIC, scalar2=None, op0=ALU.subtract
    )
    rc = sp.tile([T, 3], f32)
    nc.vector.tensor_scalar(
        out=rc[:], in0=predc[:], scalar1=MAGIC, scalar2=None, op0=ALU.add
    )
    nc.vector.tensor_scalar(
        out=rc[:], in0=rc[:], scalar1=MAGIC, scalar2=None, op0=ALU.subtract
    )

    # match mask per (t, anchor)
    m = sp.tile([T, 3], f32)
    nc.vector.tensor_scalar(
        out=m[:], in0=rr[:], scalar1=TBf[:, 2:3], scalar2=None, op0=ALU.is_equal
    )
    m2 = sp.tile([T, 3], f32)
    nc.vector.tensor_scalar(
        out=m2[:], in0=rc[:], scalar1=TBf[:, 3:4], scalar2=None, op0=ALU.is_equal
    )
    nc.vector.tensor_tensor(out=m[:], in0=m[:], in1=m2[:], op=ALU.mult)

    # contribution = |predr-tb2| + |predc-tb3| + tp*(tp-2*conf)
    ntb2 = sp.tile([T, 1], f32)
    nc.vector.tensor_scalar(
        out=ntb2[:], in0=TBf[:, 2:3], scalar1=-1.0, scalar2=None, op0=ALU.mult
    )
    ntb3 = sp.tile([T, 1], f32)
    nc.vector.tensor_scalar(
        out=ntb3[:], in0=TBf[:, 3:4], scalar1=-1.0, scalar2=None, op0=ALU.mult
    )
    d1 = sp.tile([T, 3], f32)
    nc.scalar.activation(d1[:], predr[:], ACT_F.Abs, bias=ntb2[:])
    d2 = sp.tile([T, 3], f32)
    nc.scalar.activation(d2[:], predc[:], ACT_F.Abs, bias=ntb3[:])
    nc.vector.tensor_tensor(out=d1[:], in0=d1[:], in1=d2[:], op=ALU.add)
    cf = sp.tile([T, 3], f32)
    nc.vector.tensor_scalar(
        out=cf[:], in0=gs3[:, 2, :], scalar1=-2.0, scalar2=TP[:],
        op0=ALU.mult, op1=ALU.add,
    )
    nc.vector.tensor_scalar(
        out=cf[:], in0=cf[:], scalar1=TP[:], scalar2=None, op0=ALU.mult
    )
    nc.vector.tensor_tensor(out=d1[:], in0=d1[:], in1=cf[:], op=ALU.add)
    # valid = match * keep; corr contribution = valid * d1
    nc.vector.tensor_scalar(
        out=m[:], in0=m[:], scalar1=keep[:], scalar2=None, op0=ALU.mult
    )
    nc.vector.tensor_tensor(out=m[:], in0=m[:], in1=d1[:], op=ALU.mult)
    nc.vector.tensor_reduce(
        out=ACC[0:T, NDENSE : NDENSE + 1], in_=m[:],
        axis=mybir.AxisListType.X, op=ALU.add,
    )


def _build_program(corr=True, gather=True, bcast=True, fsplit=FSPLIT,
                   dense_mode="perqueue", pe_out=True):
    import concourse.bass as bass
    import concourse.tile as tile
    from concourse import bacc, mybir

    f32 = mybir.dt.float32
    i32 = mybir.dt.int32
    ALU = mybir.AluOpType
    ACT_F = mybir.ActivationFunctionType
    ndense = NDENSE

    nc = bacc.Bacc(
        "TRN2", target_bir_lowering=False, debug=False, num_devices=N_CORES
    )
    pol = nc.dram_tensor("pol", [C, H, W], f32, kind="ExternalInput").ap()
    tb = nc.dram_tensor("tb", [T, 4], i32, kind="ExternalInput").ap()
    tp = nc.dram_tensor("tp", [T, 1], f32, kind="ExternalInput").ap()
    tri = nc.dram_tensor("tri", [T, T], f32, kind="ExternalInput").ap()
    choff = nc.dram_tensor("choff", [T, C], f32, kind="ExternalInput").ap()
    out = nc.dram_tensor("out", [1 if pe_out else 128], f32,
                         kind="ExternalOutput").ap()

    with tile.TileContext(nc) as tc:
        with (
            tc.tile_pool(name="io", bufs=3) as io,
            tc.tile_pool(name="acc", bufs=1) as accp,
            tc.tile_pool(name="small", bufs=1) as sp,
            tc.tile_pool(name="psum", bufs=1, space="PSUM") as psum,
        ):
            ACC = accp.tile([128, ndense + 1], f32)
            nc.vector.memset(ACC[:], 0.0)

            # ---------- dense loads first: one channel per DMA queue
            # (sync HWDGE / scalar HWDGE / gpsimd SWDGE) so the three
            # transfers stream in parallel ----------
            # full-channel tiles, 8 KB/partition contiguous rows (fastest
            # per-queue packet size). ch0 -> sync queue, ch1 -> scalar queue,
            # ch2 split by partition halves across both queues.
            dtiles = []
            views = [
                pol[ch].rearrange("(p a) w -> p (a w)", p=128) for ch in CONF_CH
            ]
            t0 = io.tile([128, 2048], f32, tag="in")
            nc.sync.dma_start(t0[:], views[0][:])
            t1 = io.tile([128, 2048], f32, tag="in")
            nc.scalar.dma_start(t1[:], views[1][:])
            t2 = io.tile([128, 2048], f32, tag="in")
            nc.sync.dma_start(t2[0:64, :], views[2][0:64, :])
            nc.scalar.dma_start(t2[64:128, :], views[2][64:128, :])
            dtiles = [t0, t1, t2]

            if corr:
                _build_correction(
                    nc, sp, ACC, bass, mybir, tb, tp, tri, choff, pol,
                    gather=gather, bcast=bcast,
                )

            # ---------------- dense compute: sum sigmoid(conf_ch)^2 ----------
            # ch0 (arrives first): sigmoid + Square(accum) on ACT (f32);
            # ch1, ch2: sigmoid -> bf16, square+reduce on DVE
            bf16 = mybir.dt.bfloat16
            for col, tin in enumerate(dtiles):
                if col == 0:
                    sig = io.tile([128, 2048], f32, tag="sig")
                    nc.scalar.activation(sig[:], tin[:], ACT_F.Sigmoid)
                    nc.scalar.activation(
                        tin[:], sig[:], ACT_F.Square,
                        accum_out=ACC[:, col : col + 1],
                    )
                else:
                    sigb = io.tile([128, 2048], bf16, tag="sigb")
                    nc.scalar.activation(sigb[:], tin[:], ACT_F.Sigmoid)
                    sqb = io.tile([128, 2048], bf16, tag="sqb")
                    nc.vector.tensor_tensor(
                        out=sqb[:], in0=sigb[:], in1=sigb[:], op=ALU.mult
                    )
                    nc.vector.tensor_reduce(
                        out=ACC[:, col : col + 1], in_=sqb[:],
                        axis=mybir.AxisListType.X, op=ALU.add,
                    )

            RED = sp.tile([128, 1], f32)
            nc.vector.tensor_reduce(
                out=RED[:], in_=ACC[:], axis=mybir.AxisListType.X, op=ALU.add
            )
            if pe_out:
                # cross-partition reduce on the (idle) tensor engine
                ONES = sp.tile([128, 1], f32)
                nc.vector.memset(ONES[:], 1.0)
                PS = psum.tile([1, 1], f32, space="PSUM")
                nc.tensor.matmul(out=PS[:], lhsT=RED[:], rhs=ONES[:],
                                 start=True, stop=True)
                OUTSB = sp.tile([1, 1], f32)
                nc.vector.tensor_copy(OUTSB[:], PS[:])
                nc.sync.dma_start(out[:], OUTSB[:])
            else:
                nc.sync.dma_start(out[:], RED[:])

    nc.compile()
    return nc


def get_program():
    global _PROG
    if _PROG is None:
        _PROG = _build_program()
    return _PROG


def make_in_maps(policy_output, target_boxes, target_probs):
    policy_output = np.ascontiguousarray(np.asarray(policy_output, dtype=np.float32))
    target_boxes = np.ascontiguousarray(np.asarray(target_boxes, dtype=np.int32))
    target_probs = np.ascontiguousarray(np.asarray(target_probs, dtype=np.float32))
    assert policy_output.shape == (B, C, H, W)
    in_maps = []
    for i in range(N_CORES):
        in_maps.append(
            {
                "pol": policy_output[i],
                "tb": target_boxes[i],
                "tp": target_probs[i].reshape(T, 1),
                "tri": TRI_CONST,
                "choff": CHOFF_CONST,
            }
        )
    return in_maps


def kernel(policy_output, target_boxes, target_probs):
    from concourse.bass_utils import run_bass_kernel_spmd

    nc = get_program()
    in_maps = make_in_maps(policy_output, target_boxes, target_probs)
    res = run_bass_kernel_spmd(nc, in_maps, list(range(N_CORES)))
    total = 0.0
    for i in range(N_CORES):
        total += float(res.results[i]["out"].sum(dtype=np.float64))
    return np.float32(total / DENOM)


# revision 12
# speedup vs baseline: 1.0945x; 1.0945x over previous
"""Trainium2 Bass kernel for nn_BoxDetectionLoss (8-core data parallel).

Math: reference loss = sum_{a,r,c}[ has_match ? coord+conf_loss : conf^2 ] / denom.
A pixel (r,c) can only match a target box t if r==tb[t,0] and c==tb[t,1]
(T=16 boxes per image), so the dense term is just sum sigmoid(conf_ch)^2 over
channels {2,5,8}; the match term is a correction at <=16 pixels x 3 anchors,
computed from 144 gathered elements per image.

Each of the 8 cores handles one batch image:
  - DMA the 3 conf channels (3MB), ACT sigmoid, DVE fused square+reduce.
  - indirect-DMA gather of pol[ch, tb0[t], tb1[t]] for all t, ch.
  - tiny [16,*] vector ops: pred = clip(tb + sigmoid*scale), round-half-even
    via +/- 1.5*2^23 trick, match compare, first-duplicate mask, correction
    contribution (coord + tp*(tp-2c) which equals coord + (c-tp)^2 - c^2).
  - per-partition accumulator [128, cols] -> free-dim reduce -> [128] output.
Host sums the 8x128 partials and divides by denom.
"""

import numpy as np

B, C, H, W = 8, 9, 512, 512
T = 16
N_CORES = 8
CONF_CH = (2, 5, 8)
DENOM = float(B * H * W * 3)
MAGIC = 12582912.0  # 1.5 * 2^23: x+MAGIC-MAGIC rounds to nearest-even int
FSPLIT = 1          # full-channel tiles
NDENSE = len(CONF_CH)

TRI_CONST = np.tril(np.ones((T, T), dtype=np.float32), -1)  # [t, t'] = t' < t
CHOFF_CONST = np.broadcast_to(
    (np.arange(C, dtype=np.float32) * (H * W))[None, :], (T, C)
).copy()

_PROG = None


def _build_correction_a(nc, sp, bass, mybir, tb, tp, tri, choff, pol,
                        gather=True, bcast=True):
    f32 = mybir.dt.float32
    i32 = mybir.dt.int32
    ALU = mybir.AluOpType
    ACT_F = mybir.ActivationFunctionType

    TB = sp.tile([T, 4], i32)
    nc.gpsimd.dma_start(TB[:], tb[:])
    TP = sp.tile([T, 1], f32)
    nc.gpsimd.dma_start(TP[:], tp[:])
    TRI = sp.tile([T, T], f32)
    nc.gpsimd.dma_start(TRI[:], tri[:])
    CH = sp.tile([T, C], f32)
    nc.gpsimd.dma_start(CH[:], choff[:])
    TBrep = sp.tile([T, 4 * T], i32)  # whole tb replicated per row
    if bcast:
        nc.gpsimd.dma_start(
            TBrep[:], tb.rearrange("t f -> (t f)").partition_broadcast(T)
        )
    else:
        nc.vector.memset(TBrep[:], 0)

    TBf = sp.tile([T, 4], f32)
    nc.vector.tensor_copy(TBf[:], TB[:])
    TBrepf = sp.tile([T, 4 * T], f32)
    nc.vector.tensor_copy(TBrepf[:], TBrep[:])

    # packed coords: p1 = r*512 + c, p2 = r2*512 + c2 (exact in f32)
    p1 = sp.tile([T, 1], f32)
    nc.vector.tensor_scalar(
        out=p1[:], in0=TBf[:, 0:1], scalar1=512.0, scalar2=TBf[:, 1:2],
        op0=ALU.mult, op1=ALU.add,
    )
    p2 = sp.tile([T, 1], f32)
    nc.vector.tensor_scalar(
        out=p2[:], in0=TBf[:, 2:3], scalar1=512.0, scalar2=TBf[:, 3:4],
        op0=ALU.mult, op1=ALU.add,
    )

    # row-layout packed coords of all boxes, from the replicated copy
    rep4 = TBrepf[:].rearrange("p (t f) -> p f t", f=4)
    p1row = sp.tile([T, T], f32)
    nc.vector.tensor_scalar(
        out=p1row[:], in0=rep4[:, 0, :], scalar1=512.0, scalar2=None,
        op0=ALU.mult,
    )
    nc.vector.tensor_tensor(
        out=p1row[:], in0=p1row[:], in1=rep4[:, 1, :], op=ALU.add
    )
    p2row = sp.tile([T, T], f32)
    nc.vector.tensor_scalar(
        out=p2row[:], in0=rep4[:, 2, :], scalar1=512.0, scalar2=None,
        op0=ALU.mult,
    )
    nc.vector.tensor_tensor(
        out=p2row[:], in0=p2row[:], in1=rep4[:, 3, :], op=ALU.add
    )

    # duplicate-box detection: S[t,t'] = (p1 equal) & (p2 equal), t' < t
    S = sp.tile([T, T], f32)
    nc.vector.tensor_scalar(
        out=S[:], in0=p1row[:], scalar1=p1[:], scalar2=None, op0=ALU.is_equal
    )
    S2 = sp.tile([T, T], f32)
    nc.vector.tensor_scalar(
        out=S2[:], in0=p2row[:], scalar1=p2[:], scalar2=None, op0=ALU.is_equal
    )
    nc.vector.tensor_tensor(out=S[:], in0=S[:], in1=S2[:], op=ALU.mult)
    nc.vector.tensor_tensor(out=S[:], in0=S[:], in1=TRI[:], op=ALU.mult)
    dupc = sp.tile([T, 1], f32)
    nc.vector.tensor_reduce(
        out=dupc[:], in_=S[:], axis=mybir.AxisListType.X, op=ALU.add
    )
    keep = sp.tile([T, 1], f32)
    nc.vector.tensor_scalar(
        out=keep[:], in0=dupc[:], scalar1=0.0, scalar2=None, op0=ALU.is_equal
    )

    # gather pol[ch, tb0[t], tb1[t]] for all (t, ch): offsets = ch*H*W + p1
    OFFf = sp.tile([T, C], f32)
    nc.vector.tensor_scalar(
        out=OFFf[:], in0=CH[:], scalar1=p1[:], scalar2=None, op0=ALU.add
    )
    OFFi = sp.tile([T, C], i32)
    nc.vector.tensor_copy(OFFi[:], OFFf[:])
    G = sp.tile([T, C], f32)
    if gather:
        nc.gpsimd.indirect_dma_start(
            out=G[:], out_offset=None,
            in_=pol.rearrange("c h (w a) -> (c h w) a", a=1),
            in_offset=bass.IndirectOffsetOnAxis(ap=OFFi[:], axis=0),
        )
    else:
        nc.vector.memset(G[:], 0.0)
    return dict(TB=TB, TP=TP, TBf=TBf, keep=keep, G=G)


def _build_correction_b(nc, sp, ACC, bass, mybir, ctx):
    f32 = mybir.dt.float32
    ALU = mybir.AluOpType
    ACT_F = mybir.ActivationFunctionType
    TP, TBf, keep, G = ctx["TP"], ctx["TBf"], ctx["keep"], ctx["G"]

    GS = sp.tile([T, C], f32)
    nc.scalar.activation(GS[:], G[:], ACT_F.Sigmoid)
    # channel ch = 3a + k: k=0 delta_r, k=1 delta_c, k=2 conf
    gs3 = GS[:].rearrange("p (a k) -> p k a", k=3)

    # pred = clip(tb + sigmoid*scale, 0, 511), all 3 anchors at once
    predr = sp.tile([T, 3], f32)
    nc.vector.tensor_scalar(
        out=predr[:], in0=gs3[:, 0, :], scalar1=9.0, scalar2=TBf[:, 0:1],
        op0=ALU.mult, op1=ALU.add,
    )
    nc.vector.tensor_scalar(
        out=predr[:], in0=predr[:], scalar1=511.0, scalar2=0.0,
        op0=ALU.min, op1=ALU.max,
    )
    predc = sp.tile([T, 3], f32)
    nc.vector.tensor_scalar(
        out=predc[:], in0=gs3[:, 1, :], scalar1=16.0, scalar2=TBf[:, 1:2],
        op0=ALU.mult, op1=ALU.add,
    )
    nc.vector.tensor_scalar(
        out=predc[:], in0=predc[:], scalar1=511.0, scalar2=0.0,
        op0=ALU.min, op1=ALU.max,
    )

    # round to nearest-even integer: (x + 1.5*2^23) - 1.5*2^23
    rr = sp.tile([T, 3], f32)
    nc.vector.tensor_scalar(
        out=rr[:], in0=predr[:], scalar1=MAGIC, scalar2=None, op0=ALU.add
    )
    nc.vector.tensor_scalar(
        out=rr[:], in0=rr[:], scalar1=MAGIC, scalar2=None, op0=ALU.subtract
    )
    rc = sp.tile([T, 3], f32)
    nc.vector.tensor_scalar(
        out=rc[:], in0=predc[:], scalar1=MAGIC, scalar2=None, op0=ALU.add
    )
    nc.vector.tensor_scalar(
        out=rc[:], in0=rc[:], scalar1=MAGIC, scalar2=None, op0=ALU.subtract
    )

    # match mask per (t, anchor)
    m = sp.tile([T, 3], f32)
    nc.vector.tensor_scalar(
        out=m[:], in0=rr[:], scalar1=TBf[:, 2:3], scalar2=None, op0=ALU.is_equal
    )
    m2 = sp.tile([T, 3], f32)
    nc.vector.tensor_scalar(
        out=m2[:], in0=rc[:], scalar1=TBf[:, 3:4], scalar2=None, op0=ALU.is_equal
    )
    nc.vector.tensor_tensor(out=m[:], in0=m[:], in1=m2[:], op=ALU.mult)

    # contribution = |predr-tb2| + |predc-tb3| + tp*(tp-2*conf)
    ntb2 = sp.tile([T, 1], f32)
    nc.vector.tensor_scalar(
        out=ntb2[:], in0=TBf[:, 2:3], scalar1=-1.0, scalar2=None, op0=ALU.mult
    )
    ntb3 = sp.tile([T, 1], f32)
    nc.vector.tensor_scalar(
        out=ntb3[:], in0=TBf[:, 3:4], scalar1=-1.0, scalar2=None, op0=ALU.mult
    )
    d1 = sp.tile([T, 3], f32)
    nc.scalar.activation(d1[:], predr[:], ACT_F.Abs, bias=ntb2[:])
    d2 = sp.tile([T, 3], f32)
    nc.scalar.activation(d2[:], predc[:], ACT_F.Abs, bias=ntb3[:])
    nc.vector.tensor_tensor(out=d1[:], in0=d1[:], in1=d2[:], op=ALU.add)
    cf = sp.tile([T, 3], f32)
    nc.vector.tensor_scalar(
        out=cf[:], in0=gs3[:, 2, :], scalar1=-2.0, scalar2=TP[:],
        op0=ALU.mult, op1=ALU.add,
    )
    nc.vector.tensor_scalar(
        out=cf[:], in0=cf[:], scalar1=TP[:], scalar2=None, op0=ALU.mult
    )
    nc.vector.tensor_tensor(out=d1[:], in0=d1[:], in1=cf[:], op=ALU.add)
    # valid = match * keep; corr contribution = valid * d1
    nc.vector.tensor_scalar(
        out=m[:], in0=m[:], scalar1=keep[:], scalar2=None, op0=ALU.mult
    )
    nc.vector.tensor_tensor(out=m[:], in0=m[:], in1=d1[:], op=ALU.mult)
    nc.vector.tensor_reduce(
        out=ACC[0:T, NDENSE : NDENSE + 1], in_=m[:],
        axis=mybir.AxisListType.X, op=ALU.add,
    )


def _build_program(corr=True, gather=True, bcast=True, fsplit=FSPLIT,
                   dense_mode="perqueue", pe_out=True):
    import concourse.bass as bass
    import concourse.tile as tile
    from concourse import bacc, mybir

    f32 = mybir.dt.float32
    i32 = mybir.dt.int32
    ALU = mybir.AluOpType
    ACT_F = mybir.ActivationFunctionType
    ndense = NDENSE

    nc = bacc.Bacc(
        "TRN2", target_bir_lowering=False, debug=False, num_devices=N_CORES
    )
    pol = nc.dram_tensor("pol", [C, H, W], f32, kind="ExternalInput").ap()
    tb = nc.dram_tensor("tb", [T, 4], i32, kind="ExternalInput").ap()
    tp = nc.dram_tensor("tp", [T, 1], f32, kind="ExternalInput").ap()
    tri = nc.dram_tensor("tri", [T, T], f32, kind="ExternalInput").ap()
    choff = nc.dram_tensor("choff", [T, C], f32, kind="ExternalInput").ap()
    out = nc.dram_tensor("out", [1 if pe_out else 128], f32,
                         kind="ExternalOutput").ap()

    with tile.TileContext(nc) as tc:
        with (
            tc.tile_pool(name="io", bufs=3) as io,
            tc.tile_pool(name="acc", bufs=1) as accp,
            tc.tile_pool(name="small", bufs=1) as sp,
            tc.tile_pool(name="psum", bufs=1, space="PSUM") as psum,
        ):
            ACC = accp.tile([128, ndense + 1], f32)
            nc.vector.memset(ACC[:], 0.0)

            # ---------- dense loads first: one channel per DMA queue
            # (sync HWDGE / scalar HWDGE / gpsimd SWDGE) so the three
            # transfers stream in parallel ----------
            # full-channel tiles, 8 KB/partition contiguous rows (fastest
            # per-queue packet size). ch0 -> sync queue, ch1 -> scalar queue,
            # ch2 split by partition halves across both queues.
            dtiles = []
            views = [
                pol[ch].rearrange("(p a) w -> p (a w)", p=128) for ch in CONF_CH
            ]
            t0 = io.tile([128, 2048], f32, tag="in")
            nc.sync.dma_start(t0[:], views[0][:])
            t1 = io.tile([128, 2048], f32, tag="in")
            nc.scalar.dma_start(t1[:], views[1][:])
            t2 = io.tile([128, 2048], f32, tag="in")
            nc.sync.dma_start(t2[0:64, :], views[2][0:64, :])
            nc.scalar.dma_start(t2[64:128, :], views[2][64:128, :])
            dtiles = [t0, t1, t2]

            if corr:
                corr_ctx = _build_correction_a(
                    nc, sp, bass, mybir, tb, tp, tri, choff, pol,
                    gather=gather, bcast=bcast,
                )

            # ---------------- dense compute: sum sigmoid(conf_ch)^2 ----------
            # ch0 (arrives first): sigmoid + Square(accum) on ACT (f32);
            # ch1, ch2: sigmoid -> bf16, square+reduce on DVE
            bf16 = mybir.dt.bfloat16
            for col, tin in enumerate(dtiles):
                if col == 0:
                    sig = io.tile([128, 2048], f32, tag="sig")
                    nc.scalar.activation(sig[:], tin[:], ACT_F.Sigmoid)
                    nc.scalar.activation(
                        tin[:], sig[:], ACT_F.Square,
                        accum_out=ACC[:, col : col + 1],
                    )
                else:
                    sigb = io.tile([128, 2048], bf16, tag="sigb")
                    nc.scalar.activation(sigb[:], tin[:], ACT_F.Sigmoid)
                    sqb = io.tile([128, 2048], bf16, tag="sqb")
                    nc.vector.tensor_tensor(
                        out=sqb[:], in0=sigb[:], in1=sigb[:], op=ALU.mult
                    )
                    nc.vector.tensor_reduce(
                        out=ACC[:, col : col + 1], in_=sqb[:],
                        axis=mybir.AxisListType.X, op=ALU.add,
                    )

            if corr:
                _build_correction_b(nc, sp, ACC, bass, mybir, corr_ctx)

            RED = sp.tile([128, 1], f32)
            nc.vector.tensor_reduce(
                out=RED[:], in_=ACC[:], axis=mybir.AxisListType.X, op=ALU.add
            )
            if pe_out:
                # cross-partition reduce on the (idle) tensor engine
                ONES = sp.tile([128, 1], f32)
                nc.vector.memset(ONES[:], 1.0)
                PS = psum.tile([1, 1], f32, space="PSUM")
                nc.tensor.matmul(out=PS[:], lhsT=RED[:], rhs=ONES[:],
                                 start=True, stop=True)
                OUTSB = sp.tile([1, 1], f32)
                nc.vector.tensor_copy(OUTSB[:], PS[:])
                nc.sync.dma_start(out[:], OUTSB[:])
            else:
                nc.sync.dma_start(out[:], RED[:])

    nc.compile()
    return nc


def get_program():
    global _PROG
    if _PROG is None:
        _PROG = _build_program()
    return _PROG


def make_in_maps(policy_output, target_boxes, target_probs):
    policy_output = np.ascontiguousarray(np.asarray(policy_output, dtype=np.float32))
    target_boxes = np.ascontiguousarray(np.asarray(target_boxes, dtype=np.int32))
    target_probs = np.ascontiguousarray(np.asarray(target_probs, dtype=np.float32))
    assert policy_output.shape == (B, C, H, W)
    in_maps = []
    for i in range(N_CORES):
        in_maps.append(
            {
                "pol": policy_output[i],
                "tb": target_boxes[i],
                "tp": target_probs[i].reshape(T, 1),
                "tri": TRI_CONST,
                "choff": CHOFF_CONST,
            }
        )
    return in_maps


def kernel(policy_output, target_boxes, target_probs):
    from concourse.bass_utils import run_bass_kernel_spmd

    nc = get_program()
    in_maps = make_in_maps(policy_output, target_boxes, target_probs)
    res = run_bass_kernel_spmd(nc, in_maps, list(range(N_CORES)))
    total = 0.0
    for i in range(N_CORES):
        total += float(res.results[i]["out"].sum(dtype=np.float64))
    return np.float32(total / DENOM)


# revision 13
# speedup vs baseline: 1.0980x; 1.0032x over previous
"""Trainium2 Bass kernel for nn_BoxDetectionLoss (8-core data parallel).

Math: reference loss = sum_{a,r,c}[ has_match ? coord+conf_loss : conf^2 ] / denom.
A pixel (r,c) can only match a target box t if r==tb[t,0] and c==tb[t,1]
(T=16 boxes per image), so the dense term is just sum sigmoid(conf_ch)^2 over
channels {2,5,8}; the match term is a correction at <=16 pixels x 3 anchors,
computed from 144 gathered elements per image.

Each of the 8 cores handles one batch image:
  - DMA the 3 conf channels (3MB), ACT sigmoid, DVE fused square+reduce.
  - indirect-DMA gather of pol[ch, tb0[t], tb1[t]] for all t, ch.
  - tiny [16,*] vector ops: pred = clip(tb + sigmoid*scale), round-half-even
    via +/- 1.5*2^23 trick, match compare, first-duplicate mask, correction
    contribution (coord + tp*(tp-2c) which equals coord + (c-tp)^2 - c^2).
  - per-partition accumulator [128, cols] -> free-dim reduce -> [128] output.
Host sums the 8x128 partials and divides by denom.
"""

import numpy as np

B, C, H, W = 8, 9, 512, 512
T = 16
N_CORES = 8
CONF_CH = (2, 5, 8)
DENOM = float(B * H * W * 3)
MAGIC = 12582912.0  # 1.5 * 2^23: x+MAGIC-MAGIC rounds to nearest-even int
FSPLIT = 1          # full-channel tiles
NDENSE = len(CONF_CH)

TRI_CONST = np.tril(np.ones((T, T), dtype=np.float32), -1)  # [t, t'] = t' < t
CHOFF_CONST = np.broadcast_to(
    (np.arange(C, dtype=np.float32) * (H * W))[None, :], (T, C)
).copy()

_PROG = None


def _build_correction_a(nc, sp, bass, mybir, tb, tp, tri, choff, pol,
                        gather=True, bcast=True):
    f32 = mybir.dt.float32
    i32 = mybir.dt.int32
    ALU = mybir.AluOpType
    ACT_F = mybir.ActivationFunctionType

    TB = sp.tile([T, 4], i32)
    nc.gpsimd.dma_start(TB[:], tb[:])
    TP = sp.tile([T, 1], f32)
    nc.gpsimd.dma_start(TP[:], tp[:])
    TRI = sp.tile([T, T], f32)
    nc.gpsimd.dma_start(TRI[:], tri[:])
    CH = sp.tile([T, C], f32)
    nc.gpsimd.dma_start(CH[:], choff[:])
    TBrep = sp.tile([T, 4 * T], i32)  # whole tb replicated per row
    if bcast:
        nc.gpsimd.dma_start(
            TBrep[:], tb.rearrange("t f -> (t f)").partition_broadcast(T)
        )
    else:
        nc.vector.memset(TBrep[:], 0)

    TBf = sp.tile([T, 4], f32)
    nc.vector.tensor_copy(TBf[:], TB[:])
    TBrepf = sp.tile([T, 4 * T], f32)
    nc.vector.tensor_copy(TBrepf[:], TBrep[:])

    # packed coords: p1 = r*512 + c, p2 = r2*512 + c2 (exact in f32)
    p1 = sp.tile([T, 1], f32)
    nc.vector.tensor_scalar(
        out=p1[:], in0=TBf[:, 0:1], scalar1=512.0, scalar2=TBf[:, 1:2],
        op0=ALU.mult, op1=ALU.add,
    )
    p2 = sp.tile([T, 1], f32)
    nc.vector.tensor_scalar(
        out=p2[:], in0=TBf[:, 2:3], scalar1=512.0, scalar2=TBf[:, 3:4],
        op0=ALU.mult, op1=ALU.add,
    )

    # row-layout packed coords of all boxes, from the replicated copy
    rep4 = TBrepf[:].rearrange("p (t f) -> p f t", f=4)
    p1row = sp.tile([T, T], f32)
    nc.vector.tensor_scalar(
        out=p1row[:], in0=rep4[:, 0, :], scalar1=512.0, scalar2=None,
        op0=ALU.mult,
    )
    nc.vector.tensor_tensor(
        out=p1row[:], in0=p1row[:], in1=rep4[:, 1, :], op=ALU.add
    )
    p2row = sp.tile([T, T], f32)
    nc.vector.tensor_scalar(
        out=p2row[:], in0=rep4[:, 2, :], scalar1=512.0, scalar2=None,
        op0=ALU.mult,
    )
    nc.vector.tensor_tensor(
        out=p2row[:], in0=p2row[:], in1=rep4[:, 3, :], op=ALU.add
    )

    # duplicate-box detection: S[t,t'] = (p1 equal) & (p2 equal), t' < t
    S = sp.tile([T, T], f32)
    nc.vector.tensor_scalar(
        out=S[:], in0=p1row[:], scalar1=p1[:], scalar2=None, op0=ALU.is_equal
    )
    S2 = sp.tile([T, T], f32)
    nc.vector.tensor_scalar(
        out=S2[:], in0=p2row[:], scalar1=p2[:], scalar2=None, op0=ALU.is_equal
    )
    nc.vector.tensor_tensor(out=S[:], in0=S[:], in1=S2[:], op=ALU.mult)
    nc.vector.tensor_tensor(out=S[:], in0=S[:], in1=TRI[:], op=ALU.mult)
    dupc = sp.tile([T, 1], f32)
    nc.vector.tensor_reduce(
        out=dupc[:], in_=S[:], axis=mybir.AxisListType.X, op=ALU.add
    )
    keep = sp.tile([T, 1], f32)
    nc.vector.tensor_scalar(
        out=keep[:], in0=dupc[:], scalar1=0.0, scalar2=None, op0=ALU.is_equal
    )

    # gather pol[ch, tb0[t], tb1[t]] for all (t, ch): offsets = ch*H*W + p1
    OFFf = sp.tile([T, C], f32)
    nc.vector.tensor_scalar(
        out=OFFf[:], in0=CH[:], scalar1=p1[:], scalar2=None, op0=ALU.add
    )
    OFFi = sp.tile([T, C], i32)
    nc.vector.tensor_copy(OFFi[:], OFFf[:])
    G = sp.tile([T, C], f32)
    if gather:
        nc.gpsimd.indirect_dma_start(
            out=G[:], out_offset=None,
            in_=pol.rearrange("c h (w a) -> (c h w) a", a=1),
            in_offset=bass.IndirectOffsetOnAxis(ap=OFFi[:], axis=0),
        )
    else:
        nc.vector.memset(G[:], 0.0)
    return dict(TB=TB, TP=TP, TBf=TBf, keep=keep, G=G)


def _build_correction_b(nc, sp, ACC, bass, mybir, ctx):
    f32 = mybir.dt.float32
    ALU = mybir.AluOpType
    ACT_F = mybir.ActivationFunctionType
    TP, TBf, keep, G = ctx["TP"], ctx["TBf"], ctx["keep"], ctx["G"]

    GS = sp.tile([T, C], f32)
    nc.scalar.activation(GS[:], G[:], ACT_F.Sigmoid)
    # channel ch = 3a + k: k=0 delta_r, k=1 delta_c, k=2 conf
    gs3 = GS[:].rearrange("p (a k) -> p k a", k=3)

    # pred = clip(tb + sigmoid*scale, 0, 511), all 3 anchors at once
    predr = sp.tile([T, 3], f32)
    nc.vector.tensor_scalar(
        out=predr[:], in0=gs3[:, 0, :], scalar1=9.0, scalar2=TBf[:, 0:1],
        op0=ALU.mult, op1=ALU.add,
    )
    nc.vector.tensor_scalar(
        out=predr[:], in0=predr[:], scalar1=511.0, scalar2=0.0,
        op0=ALU.min, op1=ALU.max,
    )
    predc = sp.tile([T, 3], f32)
    nc.vector.tensor_scalar(
        out=predc[:], in0=gs3[:, 1, :], scalar1=16.0, scalar2=TBf[:, 1:2],
        op0=ALU.mult, op1=ALU.add,
    )
    nc.vector.tensor_scalar(
        out=predc[:], in0=predc[:], scalar1=511.0, scalar2=0.0,
        op0=ALU.min, op1=ALU.max,
    )

    # round to nearest-even integer: (x + 1.5*2^23) - 1.5*2^23
    rr = sp.tile([T, 3], f32)
    nc.vector.tensor_scalar(
        out=rr[:], in0=predr[:], scalar1=MAGIC, scalar2=None, op0=ALU.add
    )
    nc.vector.tensor_scalar(
        out=rr[:], in0=rr[:], scalar1=MAGIC, scalar2=None, op0=ALU.subtract
    )
    rc = sp.tile([T, 3], f32)
    nc.vector.tensor_scalar(
        out=rc[:], in0=predc[:], scalar1=MAGIC, scalar2=None, op0=ALU.add
    )
    nc.vector.tensor_scalar(
        out=rc[:], in0=rc[:], scalar1=MAGIC, scalar2=None, op0=ALU.subtract
    )

    # match mask per (t, anchor)
    m = sp.tile([T, 3], f32)
    nc.vector.tensor_scalar(
        out=m[:], in0=rr[:], scalar1=TBf[:, 2:3], scalar2=None, op0=ALU.is_equal
    )
    m2 = sp.tile([T, 3], f32)
    nc.vector.tensor_scalar(
        out=m2[:], in0=rc[:], scalar1=TBf[:, 3:4], scalar2=None, op0=ALU.is_equal
    )
    nc.vector.tensor_tensor(out=m[:], in0=m[:], in1=m2[:], op=ALU.mult)

    # contribution = |predr-tb2| + |predc-tb3| + tp*(tp-2*conf)
    ntb2 = sp.tile([T, 1], f32)
    nc.vector.tensor_scalar(
        out=ntb2[:], in0=TBf[:, 2:3], scalar1=-1.0, scalar2=None, op0=ALU.mult
    )
    ntb3 = sp.tile([T, 1], f32)
    nc.vector.tensor_scalar(
        out=ntb3[:], in0=TBf[:, 3:4], scalar1=-1.0, scalar2=None, op0=ALU.mult
    )
    d1 = sp.tile([T, 3], f32)
    nc.scalar.activation(d1[:], predr[:], ACT_F.Abs, bias=ntb2[:])
    d2 = sp.tile([T, 3], f32)
    nc.scalar.activation(d2[:], predc[:], ACT_F.Abs, bias=ntb3[:])
    nc.vector.tensor_tensor(out=d1[:], in0=d1[:], in1=d2[:], op=ALU.add)
    cf = sp.tile([T, 3], f32)
    nc.vector.tensor_scalar(
        out=cf[:], in0=gs3[:, 2, :], scalar1=-2.0, scalar2=TP[:],
        op0=ALU.mult, op1=ALU.add,
    )
    nc.vector.tensor_scalar(
        out=cf[:], in0=cf[:], scalar1=TP[:], scalar2=None, op0=ALU.mult
    )
    nc.vector.tensor_tensor(out=d1[:], in0=d1[:], in1=cf[:], op=ALU.add)
    # valid = match * keep; corr contribution = valid * d1
    nc.vector.tensor_scalar(
        out=m[:], in0=m[:], scalar1=keep[:], scalar2=None, op0=ALU.mult
    )
    nc.vector.tensor_tensor(out=m[:], in0=m[:], in1=d1[:], op=ALU.mult)
    nc.vector.tensor_reduce(
        out=ACC[0:T, NDENSE : NDENSE + 1], in_=m[:],
        axis=mybir.AxisListType.X, op=ALU.add,
    )


def _build_program(corr=True, gather=True, bcast=True, fsplit=FSPLIT,
                   dense_mode="perqueue", pe_out=True):
    import concourse.bass as bass
    import concourse.tile as tile
    from concourse import bacc, mybir

    f32 = mybir.dt.float32
    i32 = mybir.dt.int32
    ALU = mybir.AluOpType
    ACT_F = mybir.ActivationFunctionType
    ndense = NDENSE

    nc = bacc.Bacc(
        "TRN2", target_bir_lowering=False, debug=False, num_devices=N_CORES
    )
    pol = nc.dram_tensor("pol", [C, H, W], f32, kind="ExternalInput").ap()
    tb = nc.dram_tensor("tb", [T, 4], i32, kind="ExternalInput").ap()
    tp = nc.dram_tensor("tp", [T, 1], f32, kind="ExternalInput").ap()
    tri = nc.dram_tensor("tri", [T, T], f32, kind="ExternalInput").ap()
    choff = nc.dram_tensor("choff", [T, C], f32, kind="ExternalInput").ap()
    out = nc.dram_tensor("out", [1 if pe_out else 128], f32,
                         kind="ExternalOutput").ap()

    with tile.TileContext(nc) as tc:
        with (
            tc.tile_pool(name="io", bufs=3) as io,
            tc.tile_pool(name="acc", bufs=1) as accp,
            tc.tile_pool(name="small", bufs=1) as sp,
            tc.tile_pool(name="psum", bufs=1, space="PSUM") as psum,
        ):
            ACC = accp.tile([128, ndense + 1], f32)
            nc.vector.memset(ACC[:], 0.0)

            # ---------- dense loads first: one channel per DMA queue
            # (sync HWDGE / scalar HWDGE / gpsimd SWDGE) so the three
            # transfers stream in parallel ----------
            # full-channel tiles, 8 KB/partition contiguous rows (fastest
            # per-queue packet size). The sync queue starts ~3.5us before the
            # scalar queue, so it carries ch0 whole (earliest compute start)
            # plus the lower halves of ch1/ch2; scalar carries upper halves.
            views = [
                pol[ch].rearrange("(p a) w -> p (a w)", p=128) for ch in CONF_CH
            ]
            t0 = io.tile([128, 2048], f32, tag="in")
            t1 = io.tile([128, 2048], f32, tag="in")
            t2 = io.tile([128, 2048], f32, tag="in")
            nc.sync.dma_start(t0[:], views[0][:])
            nc.scalar.dma_start(t1[64:128, :], views[1][64:128, :])
            nc.scalar.dma_start(t2[64:128, :], views[2][64:128, :])
            nc.sync.dma_start(t1[0:64, :], views[1][0:64, :])
            nc.sync.dma_start(t2[0:64, :], views[2][0:64, :])
            dtiles = [t0, t1, t2]

            if corr:
                corr_ctx = _build_correction_a(
                    nc, sp, bass, mybir, tb, tp, tri, choff, pol,
                    gather=gather, bcast=bcast,
                )

            # ---------------- dense compute: sum sigmoid(conf_ch)^2 ----------
            # sigmoid + Square(accum_out) both on ACT, fully f32 (exact)
            for col, tin in enumerate(dtiles):
                sig = io.tile([128, 2048], f32, tag="sig")
                nc.scalar.activation(sig[:], tin[:], ACT_F.Sigmoid)
                nc.scalar.activation(
                    tin[:], sig[:], ACT_F.Square,
                    accum_out=ACC[:, col : col + 1],
                )

            if corr:
                _build_correction_b(nc, sp, ACC, bass, mybir, corr_ctx)

            RED = sp.tile([128, 1], f32)
            nc.vector.tensor_reduce(
                out=RED[:], in_=ACC[:], axis=mybir.AxisListType.X, op=ALU.add
            )
            if pe_out:
                # cross-partition reduce on the (idle) tensor engine
                ONES = sp.tile([128, 1], f32)
                nc.vector.memset(ONES[:], 1.0)
                PS = psum.tile([1, 1], f32, space="PSUM")
                nc.tensor.matmul(out=PS[:], lhsT=RED[:], rhs=ONES[:],
                                 start=True, stop=True)
                OUTSB = sp.tile([1, 1], f32)
                nc.vector.tensor_copy(OUTSB[:], PS[:])
                nc.sync.dma_start(out[:], OUTSB[:])
            else:
                nc.sync.dma_start(out[:], RED[:])

    nc.compile()
    return nc


def get_program():
    global _PROG
    if _PROG is None:
        _PROG = _build_program()
    return _PROG


def make_in_maps(policy_output, target_boxes, target_probs):
    policy_output = np.ascontiguousarray(np.asarray(policy_output, dtype=np.float32))
    target_boxes = np.ascontiguousarray(np.asarray(target_boxes, dtype=np.int32))
    target_probs = np.ascontiguousarray(np.asarray(target_probs, dtype=np.float32))
    assert policy_output.shape == (B, C, H, W)
    in_maps = []
    for i in range(N_CORES):
        in_maps.append(
            {
                "pol": policy_output[i],
                "tb": target_boxes[i],
                "tp": target_probs[i].reshape(T, 1),
                "tri": TRI_CONST,
                "choff": CHOFF_CONST,
            }
        )
    return in_maps


def kernel(policy_output, target_boxes, target_probs):
    from concourse.bass_utils import run_bass_kernel_spmd

    nc = get_program()
    in_maps = make_in_maps(policy_output, target_boxes, target_probs)
    res = run_bass_kernel_spmd(nc, in_maps, list(range(N_CORES)))
    total = 0.0
    for i in range(N_CORES):
        total += float(res.results[i]["out"].sum(dtype=np.float64))
    return np.float32(total / DENOM)


# revision 14
# speedup vs baseline: 1.1788x; 1.0736x over previous
"""Trainium2 Bass kernel for nn_BoxDetectionLoss (8-core data parallel).

Math: reference loss = sum_{a,r,c}[ has_match ? coord+conf_loss : conf^2 ] / denom.
A pixel (r,c) can only match a target box t if r==tb[t,0] and c==tb[t,1]
(T=16 boxes per image), so the dense term is just sum sigmoid(conf_ch)^2 over
channels {2,5,8}; the match term is a correction at <=16 pixels x 3 anchors,
computed from 144 gathered elements per image.

Each of the 8 cores handles one batch image:
  - DMA the 3 conf channels (3MB), ACT sigmoid, DVE fused square+reduce.
  - indirect-DMA gather of pol[ch, tb0[t], tb1[t]] for all t, ch.
  - tiny [16,*] vector ops: pred = clip(tb + sigmoid*scale), round-half-even
    via +/- 1.5*2^23 trick, match compare, first-duplicate mask, correction
    contribution (coord + tp*(tp-2c) which equals coord + (c-tp)^2 - c^2).
  - per-partition accumulator [128, cols] -> free-dim reduce -> [128] output.
Host sums the 8x128 partials and divides by denom.
"""

import numpy as np

B, C, H, W = 8, 9, 512, 512
T = 16
N_CORES = 8
CONF_CH = (2, 5, 8)
DENOM = float(B * H * W * 3)
MAGIC = 12582912.0  # 1.5 * 2^23: x+MAGIC-MAGIC rounds to nearest-even int
FSPLIT = 1          # full-channel tiles
NDENSE = len(CONF_CH)

TRI_CONST = np.tril(np.ones((T, T), dtype=np.float32), -1)  # [t, t'] = t' < t
CHOFF_CONST = np.broadcast_to(
    (np.arange(C, dtype=np.float32) * (H * W))[None, :], (T, C)
).copy()

_PROG = None


def _build_correction_a(nc, sp, bass, mybir, tb, tp, tri, choff, pol,
                        gather=True, bcast=True):
    f32 = mybir.dt.float32
    i32 = mybir.dt.int32
    ALU = mybir.AluOpType
    ACT_F = mybir.ActivationFunctionType

    TB = sp.tile([T, 4], i32)
    nc.gpsimd.dma_start(TB[:], tb[:])
    TP = sp.tile([T, 1], f32)
    nc.gpsimd.dma_start(TP[:], tp[:])
    TRI = sp.tile([T, T], f32)
    nc.gpsimd.dma_start(TRI[:], tri[:])
    CH = sp.tile([T, C], f32)
    nc.gpsimd.dma_start(CH[:], choff[:])
    TBrep = sp.tile([T, 4 * T], i32)  # whole tb replicated per row
    if bcast:
        nc.gpsimd.dma_start(
            TBrep[:], tb.rearrange("t f -> (t f)").partition_broadcast(T)
        )
    else:
        nc.vector.memset(TBrep[:], 0)

    TBf = sp.tile([T, 4], f32)
    nc.vector.tensor_copy(TBf[:], TB[:])
    TBrepf = sp.tile([T, 4 * T], f32)
    nc.vector.tensor_copy(TBrepf[:], TBrep[:])

    # packed coords: p1 = r*512 + c, p2 = r2*512 + c2 (exact in f32)
    p1 = sp.tile([T, 1], f32)
    nc.vector.tensor_scalar(
        out=p1[:], in0=TBf[:, 0:1], scalar1=512.0, scalar2=TBf[:, 1:2],
        op0=ALU.mult, op1=ALU.add,
    )
    p2 = sp.tile([T, 1], f32)
    nc.vector.tensor_scalar(
        out=p2[:], in0=TBf[:, 2:3], scalar1=512.0, scalar2=TBf[:, 3:4],
        op0=ALU.mult, op1=ALU.add,
    )

    # row-layout packed coords of all boxes, from the replicated copy
    rep4 = TBrepf[:].rearrange("p (t f) -> p f t", f=4)
    p1row = sp.tile([T, T], f32)
    nc.vector.tensor_scalar(
        out=p1row[:], in0=rep4[:, 0, :], scalar1=512.0, scalar2=None,
        op0=ALU.mult,
    )
    nc.vector.tensor_tensor(
        out=p1row[:], in0=p1row[:], in1=rep4[:, 1, :], op=ALU.add
    )
    p2row = sp.tile([T, T], f32)
    nc.vector.tensor_scalar(
        out=p2row[:], in0=rep4[:, 2, :], scalar1=512.0, scalar2=None,
        op0=ALU.mult,
    )
    nc.vector.tensor_tensor(
        out=p2row[:], in0=p2row[:], in1=rep4[:, 3, :], op=ALU.add
    )

    # duplicate-box detection: S[t,t'] = (p1 equal) & (p2 equal), t' < t
    S = sp.tile([T, T], f32)
    nc.vector.tensor_scalar(
        out=S[:], in0=p1row[:], scalar1=p1[:], scalar2=None, op0=ALU.is_equal
    )
    S2 = sp.tile([T, T], f32)
    nc.vector.tensor_scalar(
        out=S2[:], in0=p2row[:], scalar1=p2[:], scalar2=None, op0=ALU.is_equal
    )
    nc.vector.tensor_tensor(out=S[:], in0=S[:], in1=S2[:], op=ALU.mult)
    nc.vector.tensor_tensor(out=S[:], in0=S[:], in1=TRI[:], op=ALU.mult)
    dupc = sp.tile([T, 1], f32)
    nc.vector.tensor_reduce(
        out=dupc[:], in_=S[:], axis=mybir.AxisListType.X, op=ALU.add
    )
    keep = sp.tile([T, 1], f32)
    nc.vector.tensor_scalar(
        out=keep[:], in0=dupc[:], scalar1=0.0, scalar2=None, op0=ALU.is_equal
    )

    # gather pol[ch, tb0[t], tb1[t]] for all (t, ch): offsets = ch*H*W + p1
    OFFf = sp.tile([T, C], f32)
    nc.vector.tensor_scalar(
        out=OFFf[:], in0=CH[:], scalar1=p1[:], scalar2=None, op0=ALU.add
    )
    OFFi = sp.tile([T, C], i32)
    nc.vector.tensor_copy(OFFi[:], OFFf[:])
    G = sp.tile([T, C], f32)
    if gather:
        nc.gpsimd.indirect_dma_start(
            out=G[:], out_offset=None,
            in_=pol.rearrange("c h (w a) -> (c h w) a", a=1),
            in_offset=bass.IndirectOffsetOnAxis(ap=OFFi[:], axis=0),
        )
    else:
        nc.vector.memset(G[:], 0.0)
    return dict(TB=TB, TP=TP, TBf=TBf, keep=keep, G=G)


def _build_correction_b(nc, sp, ACC, bass, mybir, ctx):
    f32 = mybir.dt.float32
    ALU = mybir.AluOpType
    ACT_F = mybir.ActivationFunctionType
    TP, TBf, keep, G = ctx["TP"], ctx["TBf"], ctx["keep"], ctx["G"]

    GS = sp.tile([T, C], f32)
    nc.scalar.activation(GS[:], G[:], ACT_F.Sigmoid)
    # channel ch = 3a + k: k=0 delta_r, k=1 delta_c, k=2 conf
    gs3 = GS[:].rearrange("p (a k) -> p k a", k=3)

    # pred = clip(tb + sigmoid*scale, 0, 511), all 3 anchors at once
    predr = sp.tile([T, 3], f32)
    nc.vector.tensor_scalar(
        out=predr[:], in0=gs3[:, 0, :], scalar1=9.0, scalar2=TBf[:, 0:1],
        op0=ALU.mult, op1=ALU.add,
    )
    nc.vector.tensor_scalar(
        out=predr[:], in0=predr[:], scalar1=511.0, scalar2=0.0,
        op0=ALU.min, op1=ALU.max,
    )
    predc = sp.tile([T, 3], f32)
    nc.vector.tensor_scalar(
        out=predc[:], in0=gs3[:, 1, :], scalar1=16.0, scalar2=TBf[:, 1:2],
        op0=ALU.mult, op1=ALU.add,
    )
    nc.vector.tensor_scalar(
        out=predc[:], in0=predc[:], scalar1=511.0, scalar2=0.0,
        op0=ALU.min, op1=ALU.max,
    )

    # round to nearest-even integer: (x + 1.5*2^23) - 1.5*2^23
    rr = sp.tile([T, 3], f32)
    nc.vector.tensor_scalar(
        out=rr[:], in0=predr[:], scalar1=MAGIC, scalar2=None, op0=ALU.add
    )
    nc.vector.tensor_scalar(
        out=rr[:], in0=rr[:], scalar1=MAGIC, scalar2=None, op0=ALU.subtract
    )
    rc = sp.tile([T, 3], f32)
    nc.vector.tensor_scalar(
        out=rc[:], in0=predc[:], scalar1=MAGIC, scalar2=None, op0=ALU.add
    )
    nc.vector.tensor_scalar(
        out=rc[:], in0=rc[:], scalar1=MAGIC, scalar2=None, op0=ALU.subtract
    )

    # match mask per (t, anchor)
    m = sp.tile([T, 3], f32)
    nc.vector.tensor_scalar(
        out=m[:], in0=rr[:], scalar1=TBf[:, 2:3], scalar2=None, op0=ALU.is_equal
    )
    m2 = sp.tile([T, 3], f32)
    nc.vector.tensor_scalar(
        out=m2[:], in0=rc[:], scalar1=TBf[:, 3:4], scalar2=None, op0=ALU.is_equal
    )
    nc.vector.tensor_tensor(out=m[:], in0=m[:], in1=m2[:], op=ALU.mult)

    # contribution = |predr-tb2| + |predc-tb3| + tp*(tp-2*conf)
    ntb2 = sp.tile([T, 1], f32)
    nc.vector.tensor_scalar(
        out=ntb2[:], in0=TBf[:, 2:3], scalar1=-1.0, scalar2=None, op0=ALU.mult
    )
    ntb3 = sp.tile([T, 1], f32)
    nc.vector.tensor_scalar(
        out=ntb3[:], in0=TBf[:, 3:4], scalar1=-1.0, scalar2=None, op0=ALU.mult
    )
    d1 = sp.tile([T, 3], f32)
    nc.scalar.activation(d1[:], predr[:], ACT_F.Abs, bias=ntb2[:])
    d2 = sp.tile([T, 3], f32)
    nc.scalar.activation(d2[:], predc[:], ACT_F.Abs, bias=ntb3[:])
    nc.vector.tensor_tensor(out=d1[:], in0=d1[:], in1=d2[:], op=ALU.add)
    cf = sp.tile([T, 3], f32)
    nc.vector.tensor_scalar(
        out=cf[:], in0=gs3[:, 2, :], scalar1=-2.0, scalar2=TP[:],
        op0=ALU.mult, op1=ALU.add,
    )
    nc.vector.tensor_scalar(
        out=cf[:], in0=cf[:], scalar1=TP[:], scalar2=None, op0=ALU.mult
    )
    nc.vector.tensor_tensor(out=d1[:], in0=d1[:], in1=cf[:], op=ALU.add)
    # valid = match * keep; corr contribution = valid * d1
    nc.vector.tensor_scalar(
        out=m[:], in0=m[:], scalar1=keep[:], scalar2=None, op0=ALU.mult
    )
    nc.vector.tensor_tensor(out=m[:], in0=m[:], in1=d1[:], op=ALU.mult)
    nc.vector.tensor_reduce(
        out=ACC[0:T, NDENSE : NDENSE + 1], in_=m[:],
        axis=mybir.AxisListType.X, op=ALU.add,
    )


def _build_program(corr=True, gather=True, bcast=True, fsplit=FSPLIT,
                   dense_mode="perqueue", pe_out=True):
    import concourse.bass as bass
    import concourse.tile as tile
    from concourse import bacc, mybir

    f32 = mybir.dt.float32
    i32 = mybir.dt.int32
    ALU = mybir.AluOpType
    ACT_F = mybir.ActivationFunctionType
    ndense = NDENSE

    nc = bacc.Bacc(
        "TRN2", target_bir_lowering=False, debug=False, num_devices=N_CORES
    )
    pol = nc.dram_tensor("pol", [C, H, W], f32, kind="ExternalInput").ap()
    tb = nc.dram_tensor("tb", [T, 4], i32, kind="ExternalInput").ap()
    tp = nc.dram_tensor("tp", [T, 1], f32, kind="ExternalInput").ap()
    tri = nc.dram_tensor("tri", [T, T], f32, kind="ExternalInput").ap()
    choff = nc.dram_tensor("choff", [T, C], f32, kind="ExternalInput").ap()
    out = nc.dram_tensor("out", [1 if pe_out else 128], f32,
                         kind="ExternalOutput").ap()

    with tile.TileContext(nc) as tc:
        with (
            tc.tile_pool(name="io", bufs=3) as io,
            tc.tile_pool(name="acc", bufs=1) as accp,
            tc.tile_pool(name="small", bufs=1) as sp,
            tc.tile_pool(name="psum", bufs=1, space="PSUM") as psum,
        ):
            ACC = accp.tile([128, ndense + 1], f32)
            nc.vector.memset(ACC[:], 0.0)

            # ---------- dense loads first: one channel per DMA queue
            # (sync HWDGE / scalar HWDGE / gpsimd SWDGE) so the three
            # transfers stream in parallel ----------
            # full-channel tiles, 8 KB/partition contiguous rows (fastest
            # per-queue packet size). The sync queue starts ~3.5us before the
            # scalar queue, so it carries ch0 whole (earliest compute start)
            # plus the lower halves of ch1/ch2; scalar carries upper halves.
            views = [
                pol[ch].rearrange("(p a) w -> p (a w)", p=128) for ch in CONF_CH
            ]
            t0 = io.tile([128, 2048], f32, tag="in")
            t1 = io.tile([128, 2048], f32, tag="in")
            t2 = io.tile([128, 2048], f32, tag="in")
            nc.sync.dma_start(t0[:], views[0][:])
            nc.scalar.dma_start(t1[:], views[1][:])
            nc.sync.dma_start(t2[0:64, :], views[2][0:64, :])
            nc.scalar.dma_start(t2[64:128, :], views[2][64:128, :])
            dtiles = [t0, t1, t2]

            if corr:
                corr_ctx = _build_correction_a(
                    nc, sp, bass, mybir, tb, tp, tri, choff, pol,
                    gather=gather, bcast=bcast,
                )

            # ---------------- dense compute: sum sigmoid(conf_ch)^2 ----------
            # ch0/ch2: sigmoid + Square(accum_out) on ACT (f32, exact);
            # ch1: bf16 sigmoid, square+reduce on DVE in the ACT shadow
            bf16 = mybir.dt.bfloat16
            for col, tin in enumerate(dtiles):
                if col == 1:
                    sigb = io.tile([128, 2048], bf16, tag="sigb")
                    nc.scalar.activation(sigb[:], tin[:], ACT_F.Sigmoid)
                    sqb = io.tile([128, 2048], bf16, tag="sqb")
                    nc.vector.tensor_tensor(
                        out=sqb[:], in0=sigb[:], in1=sigb[:], op=ALU.mult
                    )
                    nc.vector.tensor_reduce(
                        out=ACC[:, col : col + 1], in_=sqb[:],
                        axis=mybir.AxisListType.X, op=ALU.add,
                    )
                else:
                    sig = io.tile([128, 2048], f32, tag="sig")
                    nc.scalar.activation(sig[:], tin[:], ACT_F.Sigmoid)
                    nc.scalar.activation(
                        tin[:], sig[:], ACT_F.Square,
                        accum_out=ACC[:, col : col + 1],
                    )

            if corr:
                _build_correction_b(nc, sp, ACC, bass, mybir, corr_ctx)

            RED = sp.tile([128, 1], f32)
            nc.vector.tensor_reduce(
                out=RED[:], in_=ACC[:], axis=mybir.AxisListType.X, op=ALU.add
            )
            if pe_out:
                # cross-partition reduce on the (idle) tensor engine
                ONES = sp.tile([128, 1], f32)
                nc.vector.memset(ONES[:], 1.0)
                PS = psum.tile([1, 1], f32, space="PSUM")
                nc.tensor.matmul(out=PS[:], lhsT=RED[:], rhs=ONES[:],
                                 start=True, stop=True)
                OUTSB = sp.tile([1, 1], f32)
                nc.vector.tensor_copy(OUTSB[:], PS[:])
                nc.sync.dma_start(out[:], OUTSB[:])
            else:
                nc.sync.dma_start(out[:], RED[:])

    nc.compile()
    return nc


def get_program():
    global _PROG
    if _PROG is None:
        _PROG = _build_program()
    return _PROG


def make_in_maps(policy_output, target_boxes, target_probs):
    policy_output = np.ascontiguousarray(np.asarray(policy_output, dtype=np.float32))
    target_boxes = np.ascontiguousarray(np.asarray(target_boxes, dtype=np.int32))
    target_probs = np.ascontiguousarray(np.asarray(target_probs, dtype=np.float32))
    assert policy_output.shape == (B, C, H, W)
    in_maps = []
    for i in range(N_CORES):
        in_maps.append(
            {
                "pol": policy_output[i],
                "tb": target_boxes[i],
                "tp": target_probs[i].reshape(T, 1),
                "tri": TRI_CONST,
                "choff": CHOFF_CONST,
            }
        )
    return in_maps


def kernel(policy_output, target_boxes, target_probs):
    from concourse.bass_utils import run_bass_kernel_spmd

    nc = get_program()
    in_maps = make_in_maps(policy_output, target_boxes, target_probs)
    res = run_bass_kernel_spmd(nc, in_maps, list(range(N_CORES)))
    total = 0.0
    for i in range(N_CORES):
        total += float(res.results[i]["out"].sum(dtype=np.float64))
    return np.float32(total / DENOM)


# revision 15
# speedup vs baseline: 1.1944x; 1.0133x over previous
"""Trainium2 Bass kernel for nn_BoxDetectionLoss (8-core data parallel).

Math: reference loss = sum_{a,r,c}[ has_match ? coord+conf_loss : conf^2 ] / denom.
A pixel (r,c) can only match a target box t if r==tb[t,0] and c==tb[t,1]
(T=16 boxes per image), so the dense term is just sum sigmoid(conf_ch)^2 over
channels {2,5,8}; the match term is a correction at <=16 pixels x 3 anchors,
computed from 144 gathered elements per image.

Each of the 8 cores handles one batch image:
  - DMA the 3 conf channels (3MB), ACT sigmoid, DVE fused square+reduce.
  - indirect-DMA gather of pol[ch, tb0[t], tb1[t]] for all t, ch.
  - tiny [16,*] vector ops: pred = clip(tb + sigmoid*scale), round-half-even
    via +/- 1.5*2^23 trick, match compare, first-duplicate mask, correction
    contribution (coord + tp*(tp-2c) which equals coord + (c-tp)^2 - c^2).
  - per-partition accumulator [128, cols] -> free-dim reduce -> [128] output.
Host sums the 8x128 partials and divides by denom.
"""

import numpy as np

B, C, H, W = 8, 9, 512, 512
T = 16
N_CORES = 8
CONF_CH = (2, 5, 8)
DENOM = float(B * H * W * 3)
MAGIC = 12582912.0  # 1.5 * 2^23: x+MAGIC-MAGIC rounds to nearest-even int
FSPLIT = 1          # full-channel tiles
NDENSE = len(CONF_CH)

TRI_CONST = np.tril(np.ones((T, T), dtype=np.float32), -1)  # [t, t'] = t' < t
CHOFF_CONST = np.broadcast_to(
    (np.arange(C, dtype=np.float32) * (H * W))[None, :], (T, C)
).copy()

_PROG = None


def _build_correction_a(nc, sp, bass, mybir, tb, tp, tri, choff, pol,
                        gather=True, bcast=True):
    f32 = mybir.dt.float32
    i32 = mybir.dt.int32
    ALU = mybir.AluOpType
    ACT_F = mybir.ActivationFunctionType

    TB = sp.tile([T, 4], i32)
    nc.gpsimd.dma_start(TB[:], tb[:])
    TP = sp.tile([T, 1], f32)
    nc.gpsimd.dma_start(TP[:], tp[:])
    TRI = sp.tile([T, T], f32)
    nc.gpsimd.dma_start(TRI[:], tri[:])
    CH = sp.tile([T, C], f32)
    nc.gpsimd.dma_start(CH[:], choff[:])
    TBrep = sp.tile([T, 4 * T], i32)  # whole tb replicated per row
    if bcast:
        nc.gpsimd.dma_start(
            TBrep[:], tb.rearrange("t f -> (t f)").partition_broadcast(T)
        )
    else:
        nc.vector.memset(TBrep[:], 0)

    TBf = sp.tile([T, 4], f32)
    nc.vector.tensor_copy(TBf[:], TB[:])
    TBrepf = sp.tile([T, 4 * T], f32)
    nc.vector.tensor_copy(TBrepf[:], TBrep[:])

    # packed coords: p1 = r*512 + c, p2 = r2*512 + c2 (exact in f32)
    p1 = sp.tile([T, 1], f32)
    nc.vector.tensor_scalar(
        out=p1[:], in0=TBf[:, 0:1], scalar1=512.0, scalar2=TBf[:, 1:2],
        op0=ALU.mult, op1=ALU.add,
    )
    p2 = sp.tile([T, 1], f32)
    nc.vector.tensor_scalar(
        out=p2[:], in0=TBf[:, 2:3], scalar1=512.0, scalar2=TBf[:, 3:4],
        op0=ALU.mult, op1=ALU.add,
    )

    # row-layout packed coords of all boxes, from the replicated copy
    rep4 = TBrepf[:].rearrange("p (t f) -> p f t", f=4)
    p1row = sp.tile([T, T], f32)
    nc.vector.tensor_scalar(
        out=p1row[:], in0=rep4[:, 0, :], scalar1=512.0, scalar2=None,
        op0=ALU.mult,
    )
    nc.vector.tensor_tensor(
        out=p1row[:], in0=p1row[:], in1=rep4[:, 1, :], op=ALU.add
    )
    p2row = sp.tile([T, T], f32)
    nc.vector.tensor_scalar(
        out=p2row[:], in0=rep4[:, 2, :], scalar1=512.0, scalar2=None,
        op0=ALU.mult,
    )
    nc.vector.tensor_tensor(
        out=p2row[:], in0=p2row[:], in1=rep4[:, 3, :], op=ALU.add
    )

    # duplicate-box detection: S[t,t'] = (p1 equal) & (p2 equal), t' < t
    S = sp.tile([T, T], f32)
    nc.vector.tensor_scalar(
        out=S[:], in0=p1row[:], scalar1=p1[:], scalar2=None, op0=ALU.is_equal
    )
    S2 = sp.tile([T, T], f32)
    nc.vector.tensor_scalar(
        out=S2[:], in0=p2row[:], scalar1=p2[:], scalar2=None, op0=ALU.is_equal
    )
    nc.vector.tensor_tensor(out=S[:], in0=S[:], in1=S2[:], op=ALU.mult)
    nc.vector.tensor_tensor(out=S[:], in0=S[:], in1=TRI[:], op=ALU.mult)
    dupc = sp.tile([T, 1], f32)
    nc.vector.tensor_reduce(
        out=dupc[:], in_=S[:], axis=mybir.AxisListType.X, op=ALU.add
    )
    keep = sp.tile([T, 1], f32)
    nc.vector.tensor_scalar(
        out=keep[:], in0=dupc[:], scalar1=0.0, scalar2=None, op0=ALU.is_equal
    )

    # gather pol[ch, tb0[t], tb1[t]] for all (t, ch): offsets = ch*H*W + p1
    OFFf = sp.tile([T, C], f32)
    nc.vector.tensor_scalar(
        out=OFFf[:], in0=CH[:], scalar1=p1[:], scalar2=None, op0=ALU.add
    )
    OFFi = sp.tile([T, C], i32)
    nc.vector.tensor_copy(OFFi[:], OFFf[:])
    G = sp.tile([T, C], f32)
    if gather:
        nc.gpsimd.indirect_dma_start(
            out=G[:], out_offset=None,
            in_=pol.rearrange("c h (w a) -> (c h w) a", a=1),
            in_offset=bass.IndirectOffsetOnAxis(ap=OFFi[:], axis=0),
        )
    else:
        nc.vector.memset(G[:], 0.0)
    return dict(TB=TB, TP=TP, TBf=TBf, keep=keep, G=G)


def _build_correction_b(nc, sp, ACC, bass, mybir, ctx):
    f32 = mybir.dt.float32
    ALU = mybir.AluOpType
    ACT_F = mybir.ActivationFunctionType
    TP, TBf, keep, G = ctx["TP"], ctx["TBf"], ctx["keep"], ctx["G"]

    GS = sp.tile([T, C], f32)
    nc.scalar.activation(GS[:], G[:], ACT_F.Sigmoid)
    # channel ch = 3a + k: k=0 delta_r, k=1 delta_c, k=2 conf
    gs3 = GS[:].rearrange("p (a k) -> p k a", k=3)

    # pred = clip(tb + sigmoid*scale, 0, 511), all 3 anchors at once
    predr = sp.tile([T, 3], f32)
    nc.vector.tensor_scalar(
        out=predr[:], in0=gs3[:, 0, :], scalar1=9.0, scalar2=TBf[:, 0:1],
        op0=ALU.mult, op1=ALU.add,
    )
    nc.vector.tensor_scalar(
        out=predr[:], in0=predr[:], scalar1=511.0, scalar2=0.0,
        op0=ALU.min, op1=ALU.max,
    )
    predc = sp.tile([T, 3], f32)
    nc.vector.tensor_scalar(
        out=predc[:], in0=gs3[:, 1, :], scalar1=16.0, scalar2=TBf[:, 1:2],
        op0=ALU.mult, op1=ALU.add,
    )
    nc.vector.tensor_scalar(
        out=predc[:], in0=predc[:], scalar1=511.0, scalar2=0.0,
        op0=ALU.min, op1=ALU.max,
    )

    # round to nearest-even integer: (x + 1.5*2^23) - 1.5*2^23
    rr = sp.tile([T, 3], f32)
    nc.vector.tensor_scalar(
        out=rr[:], in0=predr[:], scalar1=MAGIC, scalar2=None, op0=ALU.add
    )
    nc.vector.tensor_scalar(
        out=rr[:], in0=rr[:], scalar1=MAGIC, scalar2=None, op0=ALU.subtract
    )
    rc = sp.tile([T, 3], f32)
    nc.vector.tensor_scalar(
        out=rc[:], in0=predc[:], scalar1=MAGIC, scalar2=None, op0=ALU.add
    )
    nc.vector.tensor_scalar(
        out=rc[:], in0=rc[:], scalar1=MAGIC, scalar2=None, op0=ALU.subtract
    )

    # match mask per (t, anchor)
    m = sp.tile([T, 3], f32)
    nc.vector.tensor_scalar(
        out=m[:], in0=rr[:], scalar1=TBf[:, 2:3], scalar2=None, op0=ALU.is_equal
    )
    m2 = sp.tile([T, 3], f32)
    nc.vector.tensor_scalar(
        out=m2[:], in0=rc[:], scalar1=TBf[:, 3:4], scalar2=None, op0=ALU.is_equal
    )
    nc.vector.tensor_tensor(out=m[:], in0=m[:], in1=m2[:], op=ALU.mult)

    # contribution = |predr-tb2| + |predc-tb3| + tp*(tp-2*conf)
    ntb2 = sp.tile([T, 1], f32)
    nc.vector.tensor_scalar(
        out=ntb2[:], in0=TBf[:, 2:3], scalar1=-1.0, scalar2=None, op0=ALU.mult
    )
    ntb3 = sp.tile([T, 1], f32)
    nc.vector.tensor_scalar(
        out=ntb3[:], in0=TBf[:, 3:4], scalar1=-1.0, scalar2=None, op0=ALU.mult
    )
    d1 = sp.tile([T, 3], f32)
    nc.scalar.activation(d1[:], predr[:], ACT_F.Abs, bias=ntb2[:])
    d2 = sp.tile([T, 3], f32)
    nc.scalar.activation(d2[:], predc[:], ACT_F.Abs, bias=ntb3[:])
    nc.vector.tensor_tensor(out=d1[:], in0=d1[:], in1=d2[:], op=ALU.add)
    cf = sp.tile([T, 3], f32)
    nc.vector.tensor_scalar(
        out=cf[:], in0=gs3[:, 2, :], scalar1=-2.0, scalar2=TP[:],
        op0=ALU.mult, op1=ALU.add,
    )
    nc.vector.tensor_scalar(
        out=cf[:], in0=cf[:], scalar1=TP[:], scalar2=None, op0=ALU.mult
    )
    nc.vector.tensor_tensor(out=d1[:], in0=d1[:], in1=cf[:], op=ALU.add)
    # valid = match * keep; corr contribution = valid * d1
    nc.vector.tensor_scalar(
        out=m[:], in0=m[:], scalar1=keep[:], scalar2=None, op0=ALU.mult
    )
    nc.vector.tensor_tensor(out=m[:], in0=m[:], in1=d1[:], op=ALU.mult)
    nc.vector.tensor_reduce(
        out=ACC[0:T, NDENSE : NDENSE + 1], in_=m[:],
        axis=mybir.AxisListType.X, op=ALU.add,
    )


def _build_program(corr=True, gather=True, bcast=True, fsplit=FSPLIT,
                   dense_mode="perqueue", pe_out=True):
    import concourse.bass as bass
    import concourse.tile as tile
    from concourse import bacc, mybir

    f32 = mybir.dt.float32
    i32 = mybir.dt.int32
    ALU = mybir.AluOpType
    ACT_F = mybir.ActivationFunctionType
    ndense = NDENSE

    nc = bacc.Bacc(
        "TRN2", target_bir_lowering=False, debug=False, num_devices=N_CORES
    )
    pol = nc.dram_tensor("pol", [C, H, W], f32, kind="ExternalInput").ap()
    tb = nc.dram_tensor("tb", [T, 4], i32, kind="ExternalInput").ap()
    tp = nc.dram_tensor("tp", [T, 1], f32, kind="ExternalInput").ap()
    tri = nc.dram_tensor("tri", [T, T], f32, kind="ExternalInput").ap()
    choff = nc.dram_tensor("choff", [T, C], f32, kind="ExternalInput").ap()
    out = nc.dram_tensor("out", [1 if pe_out else 128], f32,
                         kind="ExternalOutput").ap()

    with tile.TileContext(nc) as tc:
        with (
            tc.tile_pool(name="io", bufs=3) as io,
            tc.tile_pool(name="acc", bufs=1) as accp,
            tc.tile_pool(name="small", bufs=1) as sp,
            tc.tile_pool(name="psum", bufs=1, space="PSUM") as psum,
        ):
            ACC = accp.tile([128, ndense + 1], f32)
            nc.vector.memset(ACC[:], 0.0)
            ONES = sp.tile([128, 1], f32)
            nc.vector.memset(ONES[:], 1.0)

            # ---------- dense loads first: one channel per DMA queue
            # (sync HWDGE / scalar HWDGE / gpsimd SWDGE) so the three
            # transfers stream in parallel ----------
            # full-channel tiles, 8 KB/partition contiguous rows (fastest
            # per-queue packet size). The sync queue starts ~3.5us before the
            # scalar queue, so it carries ch0 whole (earliest compute start)
            # plus the lower halves of ch1/ch2; scalar carries upper halves.
            views = [
                pol[ch].rearrange("(p a) w -> p (a w)", p=128) for ch in CONF_CH
            ]
            t0 = io.tile([128, 2048], f32, tag="in")
            t1 = io.tile([128, 2048], f32, tag="in")
            t2 = io.tile([128, 2048], f32, tag="in")
            nc.sync.dma_start(t0[:], views[0][:])
            nc.scalar.dma_start(t1[:], views[1][:])
            nc.gpsimd.dma_start(t2[64:128, :], views[2][64:128, :])
            nc.sync.dma_start(t2[0:64, :], views[2][0:64, :])
            dtiles = [t0, t1, t2]

            if corr:
                corr_ctx = _build_correction_a(
                    nc, sp, bass, mybir, tb, tp, tri, choff, pol,
                    gather=gather, bcast=bcast,
                )

            # ---------------- dense compute: sum sigmoid(conf_ch)^2 ----------
            # ch0/ch2: sigmoid + Square(accum_out) on ACT (f32, exact);
            # ch1: bf16 sigmoid, square+reduce on DVE in the ACT shadow.
            # ACT order pinned: sig0, sq0, sig1, sig2, sq2 (the scheduler
            # otherwise reorders and stalls ACT on not-yet-arrived channels).
            from concourse.tile_rust import add_dep_helper

            bf16 = mybir.dt.bfloat16
            act_chain = []
            for col, tin in enumerate(dtiles):
                if col == 1:
                    sigb = io.tile([128, 2048], bf16, tag="sigb")
                    act_chain.append(
                        nc.scalar.activation(sigb[:], tin[:], ACT_F.Sigmoid)
                    )
                    sqb = io.tile([128, 2048], bf16, tag="sqb")
                    nc.vector.tensor_tensor(
                        out=sqb[:], in0=sigb[:], in1=sigb[:], op=ALU.mult
                    )
                    nc.vector.tensor_reduce(
                        out=ACC[:, col : col + 1], in_=sqb[:],
                        axis=mybir.AxisListType.X, op=ALU.add,
                    )
                else:
                    sig = io.tile([128, 2048], f32, tag="sig")
                    act_chain.append(
                        nc.scalar.activation(sig[:], tin[:], ACT_F.Sigmoid)
                    )
                    act_chain.append(
                        nc.scalar.activation(
                            tin[:], sig[:], ACT_F.Square,
                            accum_out=ACC[:, col : col + 1],
                        )
                    )
            for prev, nxt in zip(act_chain, act_chain[1:]):
                add_dep_helper(nxt.ins, prev.ins, sync=False,
                               reason="pin ACT stream order")

            if corr:
                _build_correction_b(nc, sp, ACC, bass, mybir, corr_ctx)

            RED = sp.tile([128, 1], f32)
            nc.vector.tensor_reduce(
                out=RED[:], in_=ACC[:], axis=mybir.AxisListType.X, op=ALU.add
            )
            if pe_out:
                # cross-partition reduce on the (idle) tensor engine
                PS = psum.tile([1, 1], f32, space="PSUM")
                nc.tensor.matmul(out=PS[:], lhsT=RED[:], rhs=ONES[:],
                                 start=True, stop=True)
                OUTSB = sp.tile([1, 1], f32)
                nc.vector.tensor_copy(OUTSB[:], PS[:])
                nc.sync.dma_start(out[:], OUTSB[:])
            else:
                nc.sync.dma_start(out[:], RED[:])

    nc.compile()
    return nc


def get_program():
    global _PROG
    if _PROG is None:
        _PROG = _build_program()
    return _PROG


def make_in_maps(policy_output, target_boxes, target_probs):
    policy_output = np.ascontiguousarray(np.asarray(policy_output, dtype=np.float32))
    target_boxes = np.ascontiguousarray(np.asarray(target_boxes, dtype=np.int32))
    target_probs = np.ascontiguousarray(np.asarray(target_probs, dtype=np.float32))
    assert policy_output.shape == (B, C, H, W)
    in_maps = []
    for i in range(N_CORES):
        in_maps.append(
            {
                "pol": policy_output[i],
                "tb": target_boxes[i],
                "tp": target_probs[i].reshape(T, 1),
                "tri": TRI_CONST,
                "choff": CHOFF_CONST,
            }
        )
    return in_maps


def kernel(policy_output, target_boxes, target_probs):
    from concourse.bass_utils import run_bass_kernel_spmd

    nc = get_program()
    in_maps = make_in_maps(policy_output, target_boxes, target_probs)
    res = run_bass_kernel_spmd(nc, in_maps, list(range(N_CORES)))
    total = 0.0
    for i in range(N_CORES):
        total += float(res.results[i]["out"].sum(dtype=np.float64))
    return np.float32(total / DENOM)
